# revision 5
# baseline (speedup 1.0000x reference)
"""Causal multi-head self-attention (RoPE) on 8 Trainium2 NeuronCores. v4

Sharding: core c -> batch b = c//2, head-group g = c%2 (8 of 16 heads).

v4 changes vs v3:
- Single-kick bulk loads: weights/x laid out host-side as [128, N] so each
  tensor (and each x strip) is ONE DMA (startup was 23us of serialized
  HWDGE kicks).
- Causal masks moved to GpSimd: decouples the DVE queue from the exp
  stream so pumped-in phase-1 rope runs promptly (frees PSUM rotation).
- Epilogue restructured for fast otp drain: plain copies otp->otsb +
  reciprocal straight from PSUM rows; normalization runs in-place on
  GpSimd whenever the broadcast lands (off the critical path).
- RoPE writes one [128,1024] scr tile; pair-layout conversion is ONE
  4D-AP DMA per half, kicked from the DVE queue (waits already met).
"""

import os
import sys

for _p in ("/opt/trn_rl_repo", "/root/.axon_site/_ro/trn_rl_repo"):
    if os.path.isdir(_p) and _p not in sys.path:
        sys.path.insert(0, _p)

import numpy as np
import ml_dtypes

import concourse.bass as bass
import concourse.mybir as mybir
import concourse.tile as tile
from concourse.bass_utils import run_bass_kernel_spmd

F32 = mybir.dt.float32
F32R = mybir.dt.float32r
BF16 = mybir.dt.bfloat16
AF = mybir.ActivationFunctionType

D_MODEL = 1024
NUM_HEADS = 16
HEAD_DIM = 64
BATCH = 4
SEQ = 2048
THETA = 10000.0
NCORES = 8
HG = 8              # heads per core
EG = HG * HEAD_DIM  # 512
SW = 512            # strip width (q)
KB = 128            # k block


def _split_waits(nc):
    """This walrus build accepts only one sem-wait per instruction; move
    extra waits onto wait-carrying NOPs on the same engine just before."""
    n = 0
    for fn in nc.m.functions:
        for blk in fn.blocks:
            out = []
            for inst in blk.instructions:
                si = inst.sync_info
                waits = list(si.on_wait) if si and si.on_wait else []
                if len(waits) > 1:
                    for k, w in enumerate(waits[:-1]):
                        nop = mybir.InstNoOp(
                            name=f"{inst.name}-sw{k}", ins=[], outs=[])
                        nop.engine = inst.engine
                        nop.sync_info = mybir.SyncInfo(
                            on_wait=[w], on_update=[])
                        out.append(nop)
                        n += 1
                    inst.sync_info = mybir.SyncInfo(
                        on_wait=[waits[-1]],
                        on_update=list(si.on_update or []))
                out.append(inst)
            blk.instructions = out
    return n


def build_bass(S=SEQ, split=True):
    NS = S // SW          # q strips
    nc = bass.Bass()
    # xTv[p, j*4096 + dt*512 + s] = x[j*512+s, dt*128+p]
    xTv = nc.dram_tensor("xTv", [128, NS * 8 * SW], BF16, kind="ExternalInput")
    # w*v[p, dt*512 + e] = w[dt*128+p, e]  (dims-chunk-major)
    wqv = nc.dram_tensor("wqv", [128, 8 * EG], BF16, kind="ExternalInput")
    wkv = nc.dram_tensor("wkv", [128, 8 * EG], BF16, kind="ExternalInput")
    wvv = nc.dram_tensor("wvv", [128, 8 * EG], BF16, kind="ExternalInput")
    # wov[p, et*1024 + o] = w_o.T[et*128+p, o]
    wov = nc.dram_tensor("wov", [128, 4 * D_MODEL], BF16, kind="ExternalInput")
    cosd = nc.dram_tensor("cosd", [128, S], BF16, kind="ExternalInput")
    sind = nc.dram_tensor("sind", [128, S], BF16, kind="ExternalInput")
    trid = nc.dram_tensor("trid", [128, 128], BF16, kind="ExternalInput")
    outT = nc.dram_tensor("outT", [D_MODEL, S], BF16, kind="ExternalOutput")

    with tile.TileContext(nc) as tc:
        with tc.tile_pool(name="const", bufs=1) as cpool, \
             tc.tile_pool(name="persist", bufs=1) as pers, \
             tc.tile_pool(name="work", bufs=1) as wk, \
             tc.tile_pool(name="dram", bufs=1, space="DRAM") as dpool, \
             tc.tile_pool(name="ps", bufs=1, space="PSUM") as psp:
            cos_sb = cpool.tile([128, S], BF16, tag="cos", name="cos_sb")
            sin_sb = cpool.tile([128, S], BF16, tag="sin", name="sin_sb")
            tri_sb = cpool.tile([128, 128], BF16, tag="tri", name="tri_sb")
            wq_sb = cpool.tile([128, 8 * EG], BF16, tag="wq", name="wq_sb")
            wk_sb = cpool.tile([128, 8 * EG], BF16, tag="wk", name="wk_sb")
            wv_sb = cpool.tile([128, 8 * EG], BF16, tag="wv", name="wv_sb")
            wo_sb = cpool.tile([128, 4 * D_MODEL], BF16, tag="wo", name="wo_sb")

            qt = [pers.tile([128, S], BF16, tag=f"qt{t}", name=f"qt{t}")
                  for t in range(4)]
            kt = [pers.tile([128, S], BF16, tag=f"kt{t}", name=f"kt{t}")
                  for t in range(4)]
            vts = [pers.tile([128, HG * 65], BF16, tag=f"v{i}", name=f"vt{i}")
                   for i in range(NS * 4)]
            otsb = [pers.tile([128, S], BF16, tag=f"ot{t}", name=f"otsb{t}")
                    for t in range(4)]


            # ---- constant + weight + x loads (one kick each), ordered so
            # the first phase-1 chunk (needs wq + x strip 0) starts ASAP ----
            xtiles = [wk.tile([128, 8 * SW], BF16, tag=f"xt{j}",
                              name=f"xt{j}") for j in range(NS)]
            # first chunk of phase 1 needs wq cols 0:2048 + x0 cols 0:2048;
            # split those two loads so the first matmul starts ~4us earlier
            nc.sync.dma_start(wq_sb[:, 0:4 * EG], wqv[:, 0:4 * EG])
            nc.scalar.dma_start(xtiles[0][:, 0:4 * SW], xTv[:, 0:4 * SW])
            nc.sync.dma_start(wq_sb[:, 4 * EG:8 * EG], wqv[:, 4 * EG:8 * EG])
            nc.scalar.dma_start(xtiles[0][:, 4 * SW:8 * SW],
                                xTv[:, 4 * SW:8 * SW])
            nc.sync.dma_start(wk_sb[:], wkv[:, :])
            nc.scalar.dma_start(cos_sb[:], cosd[:, :])
            nc.scalar.dma_start(sin_sb[:], sind[:, :])
            nc.sync.dma_start(xtiles[1][:], xTv[:, 8 * SW:16 * SW])
            nc.scalar.dma_start(tri_sb[:], trid[:, :])
            nc.scalar.dma_start(wv_sb[:], wvv[:, :])
            nc.sync.dma_start(xtiles[2][:], xTv[:, 16 * SW:24 * SW])
            nc.scalar.dma_start(wo_sb[:], wov[:, :])
            nc.sync.dma_start(xtiles[3][:], xTv[:, 24 * SW:32 * SW])

            # ================= phase 1 (generator per strip) =================
            def ph1(j, act_evac=False):
                js = slice(j * SW, (j + 1) * SW)
                xt = xtiles[j]
                xts = [xt[:, dt * SW:(dt + 1) * SW] for dt in range(8)]
                for wsb, dstt in ((wq_sb, qt), (wk_sb, kt)):
                    for p in range(2):
                        psE = psp.tile([128, SW], F32, tag="pp", bufs=2,
                                       name="psE")
                        psO = psp.tile([128, SW], F32, tag="pp", bufs=2,
                                       name="psO")
                        for dt in range(8):
                            wc = wsb[:, dt * EG:(dt + 1) * EG]
                            wE = wc[:, p * 128:(p + 1) * 128]
                            wO = wc[:, 256 + p * 128:256 + (p + 1) * 128]
                            nc.tensor.matmul(psE[:], wE, xts[dt],
                                             start=(dt == 0), stop=(dt == 7))
                            nc.tensor.matmul(psO[:], wO, xts[dt],
                                             start=(dt == 0), stop=(dt == 7))
                        cs = cos_sb[:, js]
                        sn = sin_sb[:, js]
                        t1 = wk.tile([128, SW], BF16, tag="tmp", bufs=8)
                        t2 = wk.tile([128, SW], BF16, tag="tmp", bufs=8)
                        t3 = wk.tile([128, SW], BF16, tag="tmp", bufs=8)
                        t4 = wk.tile([128, SW], BF16, tag="tmp", bufs=8)
                        scr = wk.tile([128, 2 * SW], BF16, tag="scr", bufs=4,
                                      name="scr")
                        if act_evac:
                            # prologue strip: ACT is idle — evacuate PSUM
                            # fast so pp rotates without waiting on DVE
                            sE = wk.tile([128, SW], BF16, tag="tmp", bufs=8,
                                         name="sE")
                            sO = wk.tile([128, SW], BF16, tag="tmp", bufs=8,
                                         name="sO")
                            nc.scalar.copy(sE[:], psE[:])
                            nc.scalar.copy(sO[:], psO[:])
                            mE, mO = sE, sO
                        else:
                            mE, mO = psE, psO
                        nc.vector.tensor_mul(t1[:], mE[:], cs)
                        nc.vector.tensor_mul(t2[:], mO[:], sn)
                        nc.vector.tensor_mul(t3[:], mO[:], cs)
                        nc.vector.tensor_mul(t4[:], mE[:], sn)
                        nc.vector.tensor_sub(scr[:, 0:SW], t1[:], t2[:])
                        nc.vector.tensor_add(scr[:, SW:2 * SW], t3[:], t4[:])
                        # pair layout: qt[t] partitions
                        # [hE 0:32, hO 32:64, h'E 64:96, h'O 96:128]
                        # one 4D-AP DMA per half (u): dst (a,b,p) part
                        # a*64+b*32+p <- scr part b0+a*32+p, col b*512+w
                        for u in range(2):
                            t = 2 * p + u
                            b0 = 64 * u
                            dst = dstt[t]
                            nc.scalar.dma_start(dst[0:32, js],
                                                scr[b0:b0 + 32, 0:SW])
                            nc.scalar.dma_start(dst[64:96, js],
                                                scr[b0 + 32:b0 + 64, 0:SW])
                            nc.gpsimd.dma_start(dst[32:64, js],
                                                scr[b0:b0 + 32, SW:2 * SW])
                            nc.gpsimd.dma_start(dst[96:128, js],
                                                scr[b0 + 32:b0 + 64,
                                                    SW:2 * SW])
                        yield
                for sb in range(4):
                    psV = psp.tile([128, EG], F32, tag="pp", bufs=2,
                                   name="psV")
                    for dt in range(8):
                        nc.tensor.matmul(
                            psV[:], xt[:, dt * SW + sb * 128:
                                       dt * SW + (sb + 1) * 128],
                            wv_sb[:, dt * EG:(dt + 1) * EG],
                            start=(dt == 0), stop=(dt == 7))
                    vt = vts[4 * j + sb]
                    vv = vt.rearrange("p (h d) -> p h d", d=65)
                    nc.scalar.copy(
                        vv[:, :, 0:64],
                        psV[:].rearrange("p (h d) -> p h d", d=64))
                    nc.scalar.copy(
                        vv[:, :, 64:65],
                        tri_sb[:, 127:128, None].broadcast_to((128, 8, 1)))
                    if sb % 2 == 1:
                        yield

            # ================= phase 3 (generator per strip) =================
            def ph3(j):
                js = slice(j * SW, (j + 1) * SW)
                for dt in range(8):
                    ds_ = slice(dt * 128, (dt + 1) * 128)
                    op = psp.tile([128, SW], F32, tag="pp", bufs=2, name="op")
                    for et in range(4):
                        nc.tensor.matmul(
                            op[:],
                            wo_sb[:, et * D_MODEL + dt * 128:
                                  et * D_MODEL + (dt + 1) * 128],
                            otsb[et][:, js],
                            start=(et == 0), stop=(et == 3))
                    ob = wk.tile([128, SW], BF16, tag="ob", bufs=4, name="ob")
                    if dt % 2 == 0:
                        nc.vector.tensor_copy(ob[:], op[:])
                        nc.sync.dma_start(outT[ds_, js], ob[:])
                    else:
                        nc.scalar.copy(ob[:], op[:])
                        nc.scalar.dma_start(outT[ds_, js], ob[:])
                    if dt % 2 == 1:
                        yield

            # ================= attention strip =================
            def attn(t, j):
                nb = 4 * j + 4
                h0, h1 = 2 * t, 2 * t + 1
                js = slice(j * SW, (j + 1) * SW)
                otp0 = psp.tile([65, SW], F32, tag="otp", bufs=2, name="otp0")
                otp1 = psp.tile([65, SW], F32, tag="otp", bufs=2, name="otp1")
                pends = []

                def emit_av(pend):
                    i, c0, w, pt = pend
                    nc.tensor.matmul(otp0[:, c0:c0 + w],
                                     vts[i][:, h0 * 65:(h0 + 1) * 65],
                                     pt[:, 0:w],
                                     start=(i == 0), stop=(i == nb - 1))
                    nc.tensor.matmul(otp1[:, c0:c0 + w],
                                     vts[i][:, h1 * 65:(h1 + 1) * 65],
                                     pt[:, SW:SW + w],
                                     start=(i == 0), stop=(i == nb - 1))

                for i in range(nb):
                    r = i - 4 * j
                    if r < 1:
                        c0, w = 0, SW
                    elif r == 1:
                        c0, w = 128, 384
                    elif r == 2:
                        c0, w = 256, 256
                    else:
                        c0, w = 384, 128
                    qs = slice(j * SW + c0, j * SW + c0 + w)
                    ks = slice(i * KB, (i + 1) * KB)
                    sc = psp.tile([128, 2 * SW], F32, tag="sc", bufs=2,
                                  name="sc")
                    nc.tensor.matmul(sc[:, 0:w], kt[t][0:64, ks],
                                     qt[t][0:64, qs], start=True, stop=True)
                    nc.tensor.matmul(sc[:, SW:SW + w], kt[t][64:128, ks],
                                     qt[t][64:128, qs], start=True, stop=True)
                    pt = wk.tile([128, 2 * SW], BF16, tag="pt", bufs=6,
                                 name="pt")
                    scv = sc.rearrange("p (g f) -> p g f", g=2)
                    ptv = pt.rearrange("p (g f) -> p g f", g=2)
                    nc.scalar.activation(ptv[:, :, 0:w], scv[:, :, 0:w],
                                         AF.Exp, scale=0.125)
                    if r >= 0:
                        nc.gpsimd.tensor_mul(
                            ptv[:, :, 0:128], ptv[:, :, 0:128],
                            tri_sb[:, None, :].broadcast_to((128, 2, 128)))
                    if len(pends) >= 4:
                        emit_av(pends.pop(0))
                    pends.append((i, c0, w, pt))
                for pn in pends:
                    emit_av(pn)

                # ---- epilogue: reciprocal from PSUM rows, PE ones-matmul
                # partition-broadcast, normalization fused into the drain ----
                dht = wk.tile([1, 2 * SW], F32, tag="dht", bufs=2,
                              name="dht")
                nc.vector.tensor_copy(dht[0:1, 0:SW], otp0[64:65, :])
                nc.vector.tensor_copy(dht[0:1, SW:2 * SW], otp1[64:65, :])
                # 1/den via Ln -> Exp(-x) on ACT (same table set as the
                # softmax Exp); Exp writes bf16 directly for the broadcast
                nc.scalar.activation(dht[:], dht[:], AF.Ln)
                drb = wk.tile([1, 2 * SW], BF16, tag="drb", bufs=2,
                              name="drb")
                nc.scalar.activation(drb[:], dht[:], AF.Exp, scale=-1.0)
                rb = psp.tile([128, SW], F32, tag="pp", bufs=2, name="rb")
                nc.tensor.matmul(rb[0:64, :], tri_sb[0:1, 0:64],
                                 drb[0:1, 0:SW], start=True, stop=True)
                nc.tensor.matmul(rb[64:128, :], tri_sb[0:1, 0:64],
                                 drb[0:1, SW:2 * SW], start=True, stop=True)
                nc.vector.tensor_copy(otsb[t][0:64, js], otp0[0:64, :])
                nc.vector.tensor_copy(otsb[t][64:128, js], otp1[0:64, :])
                nc.vector.tensor_mul(otsb[t][:, js], otsb[t][:, js], rb[:])

            # ================= main schedule =================
            def drain(g):
                for _ in g:
                    pass

            def pump(g, n):
                if g is None:
                    return None
                for _ in range(n):
                    try:
                        next(g)
                    except StopIteration:
                        return None
                return g

            ph1_gens = [ph1(j, act_evac=(j == 0)) for j in range(NS)]
            ph3_gens = [ph3(j) for j in range(NS)]
            drain(ph1_gens[0])
            ph1_gens[0] = None
            for j in range(NS):
                for t in range(4):
                    attn(t, j)
                    if j + 1 < NS:
                        ph1_gens[j + 1] = pump(ph1_gens[j + 1], 2)
                    if j >= 1:
                        ph3_gens[j - 1] = pump(ph3_gens[j - 1], 1)
                if j + 1 < NS and ph1_gens[j + 1] is not None:
                    drain(ph1_gens[j + 1])
                    ph1_gens[j + 1] = None
            for g in ph3_gens:
                if g is not None:
                    drain(g)
    if split:
        _split_waits(nc)
    return nc


def _rope_tables(S):
    inv = 1.0 / (THETA ** (np.arange(0, HEAD_DIM, 2, dtype=np.float64)
                           / HEAD_DIM))
    pos = np.arange(S, dtype=np.float64)
    fr = np.outer(pos, inv)
    return np.cos(fr).astype(np.float32), np.sin(fr).astype(np.float32)


def make_inputs(x, w_q, w_k, w_v, w_o, cos, sin, S=SEQ):
    """Build the 8 per-core input dicts (host-side shard + layout prep)."""
    bf = ml_dtypes.bfloat16
    NS = S // SW
    cosT = np.ascontiguousarray(cos[:S].T)             # [32, S]
    sinT = np.ascontiguousarray(sin[:S].T)
    cosd = np.ascontiguousarray(np.tile(cosT, (4, 1))).astype(bf)
    sind = np.ascontiguousarray(np.tile(sinT, (4, 1))).astype(bf)
    trid = (np.arange(128)[None, :] >= np.arange(128)[:, None]).astype(bf)

    def chunk128(wT, inner):
        # wT [1024, inner] -> [128, 8*inner]: out[p, d*inner+e] = wT[d*128+p, e]
        return np.ascontiguousarray(
            wT.reshape(8, 128, inner).transpose(1, 0, 2).reshape(128, 8 * inner))

    xTvs = []
    for b in range(x.shape[0]):
        xb = x[b]                                      # [S, D]
        # [128, NS*8*SW]: out[p, j*8*SW + dt*SW + s] = x[j*SW+s, dt*128+p]
        v = xb.reshape(NS, SW, 8, 128).transpose(3, 0, 2, 1).reshape(
            128, NS * 8 * SW)
        xTvs.append(np.ascontiguousarray(v).astype(bf))

    per_g = {}
    for g in range(2):
        perm = []
        for par in (0, 1):  # 0 -> evens, 1 -> odds
            for blk in range(2):
                for h in range(4):
                    gh = g * 8 + blk * 4 + h
                    perm += [gh * 64 + 2 * i + par for i in range(32)]
        perm = np.asarray(perm)
        es = slice(g * EG, (g + 1) * EG)
        wqT = w_q[perm, :].T                           # [1024, 512]
        wkT = w_k[perm, :].T
        wvT = w_v[es, :].T
        woT = w_o[:, es].T                             # [512, 1024]
        wov = woT.reshape(4, 128, D_MODEL).transpose(1, 0, 2).reshape(
            128, 4 * D_MODEL)
        per_g[g] = dict(
            wqv=chunk128(wqT, EG).astype(bf),
            wkv=chunk128(wkT, EG).astype(bf),
            wvv=chunk128(wvT, EG).astype(bf),
            wov=np.ascontiguousarray(wov).astype(bf),
        )
    in_maps = []
    for c in range(NCORES):
        b, g = c // 2, c % 2
        m = dict(xTv=xTvs[b], cosd=cosd, sind=sind, trid=trid, **per_g[g])
        in_maps.append(m)
    return in_maps


_CACHE = {}
LAST_RESULTS = None


def kernel(x, w_q, w_k, w_v, w_o, cos, sin):
    global LAST_RESULTS
    x = np.asarray(x)
    S = x.shape[1]
    in_maps = make_inputs(np.asarray(x), np.asarray(w_q), np.asarray(w_k),
                          np.asarray(w_v), np.asarray(w_o),
                          np.asarray(cos), np.asarray(sin), S=S)
    if S not in _CACHE:
        _CACHE[S] = build_bass(S=S)
    nc = _CACHE[S]
    res = run_bass_kernel_spmd(nc, in_maps, core_ids=list(range(NCORES)))
    LAST_RESULTS = res
    outs = [np.asarray(r["outT"], dtype=np.float32) for r in res.results]
    full = np.stack(
        [(outs[2 * b] + outs[2 * b + 1]).T for b in range(x.shape[0])], axis=0)
    return np.ascontiguousarray(full).astype(np.float32)


# revision 8
# speedup vs baseline: 1.0174x; 1.0174x over previous
"""Causal multi-head self-attention (RoPE) on 8 Trainium2 NeuronCores. v4

Sharding: core c -> batch b = c//2, head-group g = c%2 (8 of 16 heads).

v4 changes vs v3:
- Single-kick bulk loads: weights/x laid out host-side as [128, N] so each
  tensor (and each x strip) is ONE DMA (startup was 23us of serialized
  HWDGE kicks).
- Causal masks moved to GpSimd: decouples the DVE queue from the exp
  stream so pumped-in phase-1 rope runs promptly (frees PSUM rotation).
- Epilogue restructured for fast otp drain: plain copies otp->otsb +
  reciprocal straight from PSUM rows; normalization runs in-place on
  GpSimd whenever the broadcast lands (off the critical path).
- RoPE writes one [128,1024] scr tile; pair-layout conversion is ONE
  4D-AP DMA per half, kicked from the DVE queue (waits already met).
"""

import os
import sys

for _p in ("/opt/trn_rl_repo", "/root/.axon_site/_ro/trn_rl_repo"):
    if os.path.isdir(_p) and _p not in sys.path:
        sys.path.insert(0, _p)

import numpy as np
import ml_dtypes

import concourse.bass as bass
import concourse.mybir as mybir
import concourse.tile as tile
from concourse.bass_utils import run_bass_kernel_spmd

F32 = mybir.dt.float32
F32R = mybir.dt.float32r
BF16 = mybir.dt.bfloat16
AF = mybir.ActivationFunctionType

D_MODEL = 1024
NUM_HEADS = 16
HEAD_DIM = 64
BATCH = 4
SEQ = 2048
THETA = 10000.0
NCORES = 8
HG = 8              # heads per core
EG = HG * HEAD_DIM  # 512
SW = 512            # strip width (q)
KB = 128            # k block


def _split_waits(nc):
    """This walrus build accepts only one sem-wait per instruction; move
    extra waits onto wait-carrying NOPs on the same engine just before."""
    n = 0
    for fn in nc.m.functions:
        for blk in fn.blocks:
            out = []
            for inst in blk.instructions:
                si = inst.sync_info
                waits = list(si.on_wait) if si and si.on_wait else []
                if len(waits) > 1:
                    for k, w in enumerate(waits[:-1]):
                        nop = mybir.InstNoOp(
                            name=f"{inst.name}-sw{k}", ins=[], outs=[])
                        nop.engine = inst.engine
                        nop.sync_info = mybir.SyncInfo(
                            on_wait=[w], on_update=[])
                        out.append(nop)
                        n += 1
                    inst.sync_info = mybir.SyncInfo(
                        on_wait=[waits[-1]],
                        on_update=list(si.on_update or []))
                out.append(inst)
            blk.instructions = out
    return n


def build_bass(S=SEQ, split=True):
    NS = S // SW          # q strips
    nc = bass.Bass()
    # xTv[p, j*4096 + dt*512 + s] = x[j*512+s, dt*128+p]
    xTv = nc.dram_tensor("xTv", [128, NS * 8 * SW], BF16, kind="ExternalInput")
    # w*v[p, dt*512 + e] = w[dt*128+p, e]  (dims-chunk-major)
    wqv = nc.dram_tensor("wqv", [128, 8 * EG], BF16, kind="ExternalInput")
    wkv = nc.dram_tensor("wkv", [128, 8 * EG], BF16, kind="ExternalInput")
    wvv = nc.dram_tensor("wvv", [128, 8 * EG], BF16, kind="ExternalInput")
    # wov[p, et*1024 + o] = w_o.T[et*128+p, o]
    wov = nc.dram_tensor("wov", [128, 4 * D_MODEL], BF16, kind="ExternalInput")
    cosd = nc.dram_tensor("cosd", [128, S], BF16, kind="ExternalInput")
    sind = nc.dram_tensor("sind", [128, S], BF16, kind="ExternalInput")
    trid = nc.dram_tensor("trid", [128, 128], BF16, kind="ExternalInput")
    outT = nc.dram_tensor("outT", [D_MODEL, S], BF16, kind="ExternalOutput")

    with tile.TileContext(nc) as tc:
        with tc.tile_pool(name="const", bufs=1) as cpool, \
             tc.tile_pool(name="persist", bufs=1) as pers, \
             tc.tile_pool(name="work", bufs=1) as wk, \
             tc.tile_pool(name="dram", bufs=1, space="DRAM") as dpool, \
             tc.tile_pool(name="ps", bufs=1, space="PSUM") as psp:
            cos_sb = cpool.tile([128, S], BF16, tag="cos", name="cos_sb")
            sin_sb = cpool.tile([128, S], BF16, tag="sin", name="sin_sb")
            tri_sb = cpool.tile([128, 128], BF16, tag="tri", name="tri_sb")
            wq_sb = cpool.tile([128, 8 * EG], BF16, tag="wq", name="wq_sb")
            wk_sb = cpool.tile([128, 8 * EG], BF16, tag="wk", name="wk_sb")
            wv_sb = cpool.tile([128, 8 * EG], BF16, tag="wv", name="wv_sb")
            wo_sb = cpool.tile([128, 4 * D_MODEL], BF16, tag="wo", name="wo_sb")

            qt = [pers.tile([128, S], BF16, tag=f"qt{t}", name=f"qt{t}")
                  for t in range(4)]
            kt = [pers.tile([128, S], BF16, tag=f"kt{t}", name=f"kt{t}")
                  for t in range(4)]
            vts = [pers.tile([128, HG * 65], BF16, tag=f"v{i}", name=f"vt{i}")
                   for i in range(NS * 4)]
            otsb = [pers.tile([128, S], BF16, tag=f"ot{t}", name=f"otsb{t}")
                    for t in range(4)]


            # ---- constant + weight + x loads (one kick each), ordered so
            # the first phase-1 chunk (needs wq + x strip 0) starts ASAP ----
            xtiles = [wk.tile([128, 8 * SW], BF16, tag=f"xt{j}",
                              name=f"xt{j}") for j in range(NS)]
            # first chunk of phase 1 needs wq cols 0:2048 + x0 cols 0:2048;
            # split those two loads so the first matmul starts ~4us earlier
            nc.sync.dma_start(wq_sb[:, 0:4 * EG], wqv[:, 0:4 * EG])
            nc.scalar.dma_start(xtiles[0][:, 0:4 * SW], xTv[:, 0:4 * SW])
            nc.sync.dma_start(wq_sb[:, 4 * EG:8 * EG], wqv[:, 4 * EG:8 * EG])
            nc.scalar.dma_start(xtiles[0][:, 4 * SW:8 * SW],
                                xTv[:, 4 * SW:8 * SW])
            nc.sync.dma_start(wk_sb[:], wkv[:, :])
            nc.scalar.dma_start(cos_sb[:], cosd[:, :])
            nc.scalar.dma_start(sin_sb[:], sind[:, :])
            nc.sync.dma_start(xtiles[1][:], xTv[:, 8 * SW:16 * SW])
            nc.scalar.dma_start(tri_sb[:], trid[:, :])
            nc.scalar.dma_start(wv_sb[:], wvv[:, :])
            nc.sync.dma_start(xtiles[2][:], xTv[:, 16 * SW:24 * SW])
            nc.scalar.dma_start(wo_sb[:], wov[:, :])
            nc.sync.dma_start(xtiles[3][:], xTv[:, 24 * SW:32 * SW])

            # ================= phase 1 (generator per strip) =================
            def ph1(j, act_evac=False):
                js = slice(j * SW, (j + 1) * SW)
                xt = xtiles[j]
                xts = [xt[:, dt * SW:(dt + 1) * SW] for dt in range(8)]
                for wsb, dstt in ((wq_sb, qt), (wk_sb, kt)):
                    for p in range(2):
                        psE = psp.tile([128, SW], F32, tag="pp", bufs=2,
                                       name="psE")
                        psO = psp.tile([128, SW], F32, tag="pp", bufs=2,
                                       name="psO")
                        for dt in range(8):
                            wc = wsb[:, dt * EG:(dt + 1) * EG]
                            wE = wc[:, p * 128:(p + 1) * 128]
                            wO = wc[:, 256 + p * 128:256 + (p + 1) * 128]
                            nc.tensor.matmul(psE[:], wE, xts[dt],
                                             start=(dt == 0), stop=(dt == 7))
                            nc.tensor.matmul(psO[:], wO, xts[dt],
                                             start=(dt == 0), stop=(dt == 7))
                        cs = cos_sb[:, js]
                        sn = sin_sb[:, js]
                        t1 = wk.tile([128, SW], BF16, tag="tmp", bufs=8)
                        t2 = wk.tile([128, SW], BF16, tag="tmp", bufs=8)
                        t3 = wk.tile([128, SW], BF16, tag="tmp", bufs=8)
                        t4 = wk.tile([128, SW], BF16, tag="tmp", bufs=8)
                        scr = wk.tile([128, 2 * SW], BF16, tag="scr", bufs=4,
                                      name="scr")
                        if act_evac:
                            # prologue strip: ACT is idle — evacuate PSUM
                            # fast so pp rotates without waiting on DVE
                            sE = wk.tile([128, SW], BF16, tag="tmp", bufs=8,
                                         name="sE")
                            sO = wk.tile([128, SW], BF16, tag="tmp", bufs=8,
                                         name="sO")
                            nc.scalar.copy(sE[:], psE[:])
                            nc.scalar.copy(sO[:], psO[:])
                            mE, mO = sE, sO
                        else:
                            mE, mO = psE, psO
                        nc.vector.tensor_mul(t1[:], mE[:], cs)
                        nc.vector.tensor_mul(t2[:], mO[:], sn)
                        nc.vector.tensor_mul(t3[:], mO[:], cs)
                        nc.vector.tensor_mul(t4[:], mE[:], sn)
                        nc.vector.tensor_sub(scr[:, 0:SW], t1[:], t2[:])
                        nc.vector.tensor_add(scr[:, SW:2 * SW], t3[:], t4[:])
                        # pair layout: qt[t] partitions
                        # [hE 0:32, hO 32:64, h'E 64:96, h'O 96:128]
                        # one 4D-AP DMA per half (u): dst (a,b,p) part
                        # a*64+b*32+p <- scr part b0+a*32+p, col b*512+w
                        for u in range(2):
                            t = 2 * p + u
                            b0 = 64 * u
                            dst = dstt[t]
                            nc.sync.dma_start(dst[0:32, js],
                                               scr[b0:b0 + 32, 0:SW])
                            nc.sync.dma_start(dst[64:96, js],
                                              scr[b0 + 32:b0 + 64, 0:SW])
                            nc.sync.dma_start(dst[32:64, js],
                                              scr[b0:b0 + 32, SW:2 * SW])
                            nc.sync.dma_start(dst[96:128, js],
                                              scr[b0 + 32:b0 + 64,
                                                  SW:2 * SW])
                        yield
                for sb in range(4):
                    psV = psp.tile([128, EG], F32, tag="pp", bufs=2,
                                   name="psV")
                    for dt in range(8):
                        nc.tensor.matmul(
                            psV[:], xt[:, dt * SW + sb * 128:
                                       dt * SW + (sb + 1) * 128],
                            wv_sb[:, dt * EG:(dt + 1) * EG],
                            start=(dt == 0), stop=(dt == 7))
                    vt = vts[4 * j + sb]
                    vv = vt.rearrange("p (h d) -> p h d", d=65)
                    nc.scalar.copy(
                        vv[:, :, 0:64],
                        psV[:].rearrange("p (h d) -> p h d", d=64))
                    nc.scalar.copy(
                        vv[:, :, 64:65],
                        tri_sb[:, 127:128, None].broadcast_to((128, 8, 1)))
                    if sb % 2 == 1:
                        yield

            # ================= phase 3 (generator per strip) =================
            def ph3(j):
                js = slice(j * SW, (j + 1) * SW)
                for dt in range(8):
                    ds_ = slice(dt * 128, (dt + 1) * 128)
                    op = psp.tile([128, SW], F32, tag="pp", bufs=2, name="op")
                    for et in range(4):
                        nc.tensor.matmul(
                            op[:],
                            wo_sb[:, et * D_MODEL + dt * 128:
                                  et * D_MODEL + (dt + 1) * 128],
                            otsb[et][:, js],
                            start=(et == 0), stop=(et == 3))
                    ob = wk.tile([128, SW], BF16, tag="ob", bufs=4, name="ob")
                    if dt % 2 == 0:
                        nc.vector.tensor_copy(ob[:], op[:])
                    else:
                        nc.scalar.copy(ob[:], op[:])
                    nc.sync.dma_start(outT[ds_, js], ob[:])
                    if dt % 2 == 1:
                        yield

            # ================= attention strip =================
            def attn(t, j):
                nb = 4 * j + 4
                h0, h1 = 2 * t, 2 * t + 1
                js = slice(j * SW, (j + 1) * SW)
                otp0 = psp.tile([65, SW], F32, tag="otp", bufs=2, name="otp0")
                otp1 = psp.tile([65, SW], F32, tag="otp", bufs=2, name="otp1")
                pends = []

                def emit_av(pend):
                    i, c0, w, pt = pend
                    nc.tensor.matmul(otp0[:, c0:c0 + w],
                                     vts[i][:, h0 * 65:(h0 + 1) * 65],
                                     pt[:, 0:w],
                                     start=(i == 0), stop=(i == nb - 1))
                    nc.tensor.matmul(otp1[:, c0:c0 + w],
                                     vts[i][:, h1 * 65:(h1 + 1) * 65],
                                     pt[:, SW:SW + w],
                                     start=(i == 0), stop=(i == nb - 1))

                for i in range(nb):
                    if i == 4:
                        yield
                    r = i - 4 * j
                    if r < 1:
                        c0, w = 0, SW
                    elif r == 1:
                        c0, w = 128, 384
                    elif r == 2:
                        c0, w = 256, 256
                    else:
                        c0, w = 384, 128
                    qs = slice(j * SW + c0, j * SW + c0 + w)
                    ks = slice(i * KB, (i + 1) * KB)
                    sc = psp.tile([128, 2 * SW], F32, tag="sc", bufs=2,
                                  name="sc")
                    nc.tensor.matmul(sc[:, 0:w], kt[t][0:64, ks],
                                     qt[t][0:64, qs], start=True, stop=True)
                    nc.tensor.matmul(sc[:, SW:SW + w], kt[t][64:128, ks],
                                     qt[t][64:128, qs], start=True, stop=True)
                    pt = wk.tile([128, 2 * SW], BF16, tag="pt", bufs=6,
                                 name="pt")
                    scv = sc.rearrange("p (g f) -> p g f", g=2)
                    ptv = pt.rearrange("p (g f) -> p g f", g=2)
                    nc.scalar.activation(ptv[:, :, 0:w], scv[:, :, 0:w],
                                         AF.Exp, scale=0.125)
                    if r >= 0:
                        nc.gpsimd.tensor_mul(
                            ptv[:, :, 0:128], ptv[:, :, 0:128],
                            tri_sb[:, None, :].broadcast_to((128, 2, 128)))
                    if len(pends) >= 4:
                        emit_av(pends.pop(0))
                    pends.append((i, c0, w, pt))
                for pn in pends:
                    emit_av(pn)

                # ---- epilogue (emitted staggered, after the next strip
                # has started, so Ln/Exp do not block its exp stream) ----
                yield
                dht = wk.tile([1, 2 * SW], F32, tag="dht", bufs=2,
                              name="dht")
                nc.vector.tensor_copy(dht[0:1, 0:SW], otp0[64:65, :])
                nc.vector.tensor_copy(dht[0:1, SW:2 * SW], otp1[64:65, :])
                # 1/den via Ln -> Exp(-x) on ACT (same table set as the
                # softmax Exp); Exp writes bf16 directly for the broadcast
                nc.scalar.activation(dht[:], dht[:], AF.Ln)
                drb = wk.tile([1, 2 * SW], BF16, tag="drb", bufs=2,
                              name="drb")
                nc.scalar.activation(drb[:], dht[:], AF.Exp, scale=-1.0)
                rb = psp.tile([128, SW], F32, tag="pp", bufs=2, name="rb")
                nc.tensor.matmul(rb[0:64, :], tri_sb[0:1, 0:64],
                                 drb[0:1, 0:SW], start=True, stop=True)
                nc.tensor.matmul(rb[64:128, :], tri_sb[0:1, 0:64],
                                 drb[0:1, SW:2 * SW], start=True, stop=True)
                nc.vector.tensor_copy(otsb[t][0:64, js], otp0[0:64, :])
                nc.vector.tensor_copy(otsb[t][64:128, js], otp1[0:64, :])
                nc.vector.tensor_mul(otsb[t][:, js], otsb[t][:, js], rb[:])

            # ================= main schedule =================
            def drain(g):
                for _ in g:
                    pass

            def pump(g, n):
                if g is None:
                    return None
                for _ in range(n):
                    try:
                        next(g)
                    except StopIteration:
                        return None
                return g

            ph1_gens = [ph1(j, act_evac=(j == 0)) for j in range(NS)]
            ph3_gens = [ph3(j) for j in range(NS)]
            drain(ph1_gens[0])
            ph1_gens[0] = None
            # strip order: j0, j1, then j2/j3 interleaved so the ACT-bound
            # j3 strips are padded with PE work from j2 strips + phase 3
            order = [(t, 0) for t in range(4)] + [(t, 1) for t in range(4)] \
                + [(0, 2), (1, 2), (0, 3), (2, 2), (1, 3), (3, 2), (2, 3),
                   (3, 3)]
            pump_after = {
                0: [(1, 1, 2)], 1: [(1, 1, 2)], 2: [(1, 1, 2)],
                3: [(1, 1, 2)],
                4: [(1, 2, 2)], 5: [(1, 2, 2)], 6: [(1, 2, 2)],
                7: [(1, 3, 2)], 8: [(1, 3, 2)], 9: [(1, 3, 2)],
                10: [(3, 0, 2)], 11: [(3, 0, 2)],
                12: [(3, 1, 2)], 13: [(3, 1, 2)],
                14: [(3, 2, 2)], 15: [(3, 2, 2)],
            }
            prev_epi = None
            for si, (t, j) in enumerate(order):
                g = attn(t, j)
                next(g)                # blocks 0..3
                if prev_epi is not None:
                    pump(prev_epi, 1)  # previous strip's epilogue
                for _ in g:            # remaining blocks
                    break
                prev_epi_new = g       # paused at pre-epilogue yield
                if prev_epi is not None:
                    drain(prev_epi)
                prev_epi = prev_epi_new
                for kind, idx, n in pump_after.get(si, []):
                    if kind == 1:
                        ph1_gens[idx] = pump(ph1_gens[idx], n)
                    else:
                        ph3_gens[idx] = pump(ph3_gens[idx], n)
            if prev_epi is not None:
                drain(prev_epi)
            for g in ph3_gens:
                if g is not None:
                    drain(g)
    if split:
        _split_waits(nc)
    return nc


def _rope_tables(S):
    inv = 1.0 / (THETA ** (np.arange(0, HEAD_DIM, 2, dtype=np.float64)
                           / HEAD_DIM))
    pos = np.arange(S, dtype=np.float64)
    fr = np.outer(pos, inv)
    return np.cos(fr).astype(np.float32), np.sin(fr).astype(np.float32)


def make_inputs(x, w_q, w_k, w_v, w_o, cos, sin, S=SEQ):
    """Build the 8 per-core input dicts (host-side shard + layout prep)."""
    bf = ml_dtypes.bfloat16
    NS = S // SW
    cosT = np.ascontiguousarray(cos[:S].T)             # [32, S]
    sinT = np.ascontiguousarray(sin[:S].T)
    cosd = np.ascontiguousarray(np.tile(cosT, (4, 1))).astype(bf)
    sind = np.ascontiguousarray(np.tile(sinT, (4, 1))).astype(bf)
    trid = (np.arange(128)[None, :] >= np.arange(128)[:, None]).astype(bf)

    def chunk128(wT, inner):
        # wT [1024, inner] -> [128, 8*inner]: out[p, d*inner+e] = wT[d*128+p, e]
        return np.ascontiguousarray(
            wT.reshape(8, 128, inner).transpose(1, 0, 2).reshape(128, 8 * inner))

    xTvs = []
    for b in range(x.shape[0]):
        xb = x[b]                                      # [S, D]
        # [128, NS*8*SW]: out[p, j*8*SW + dt*SW + s] = x[j*SW+s, dt*128+p]
        v = xb.reshape(NS, SW, 8, 128).transpose(3, 0, 2, 1).reshape(
            128, NS * 8 * SW)
        xTvs.append(np.ascontiguousarray(v).astype(bf))

    per_g = {}
    for g in range(2):
        perm = []
        for par in (0, 1):  # 0 -> evens, 1 -> odds
            for blk in range(2):
                for h in range(4):
                    gh = g * 8 + blk * 4 + h
                    perm += [gh * 64 + 2 * i + par for i in range(32)]
        perm = np.asarray(perm)
        es = slice(g * EG, (g + 1) * EG)
        wqT = w_q[perm, :].T                           # [1024, 512]
        wkT = w_k[perm, :].T
        wvT = w_v[es, :].T
        woT = w_o[:, es].T                             # [512, 1024]
        wov = woT.reshape(4, 128, D_MODEL).transpose(1, 0, 2).reshape(
            128, 4 * D_MODEL)
        per_g[g] = dict(
            wqv=chunk128(wqT, EG).astype(bf),
            wkv=chunk128(wkT, EG).astype(bf),
            wvv=chunk128(wvT, EG).astype(bf),
            wov=np.ascontiguousarray(wov).astype(bf),
        )
    in_maps = []
    for c in range(NCORES):
        b, g = c // 2, c % 2
        m = dict(xTv=xTvs[b], cosd=cosd, sind=sind, trid=trid, **per_g[g])
        in_maps.append(m)
    return in_maps


_CACHE = {}
LAST_RESULTS = None


def kernel(x, w_q, w_k, w_v, w_o, cos, sin):
    global LAST_RESULTS
    x = np.asarray(x)
    S = x.shape[1]
    in_maps = make_inputs(np.asarray(x), np.asarray(w_q), np.asarray(w_k),
                          np.asarray(w_v), np.asarray(w_o),
                          np.asarray(cos), np.asarray(sin), S=S)
    if S not in _CACHE:
        _CACHE[S] = build_bass(S=S)
    nc = _CACHE[S]
    res = run_bass_kernel_spmd(nc, in_maps, core_ids=list(range(NCORES)))
    LAST_RESULTS = res
    outs = [np.asarray(r["outT"], dtype=np.float32) for r in res.results]
    full = np.stack(
        [(outs[2 * b] + outs[2 * b + 1]).T for b in range(x.shape[0])], axis=0)
    return np.ascontiguousarray(full).astype(np.float32)
